# revision 4
# baseline (speedup 1.0000x reference)
"""Associative-embedding (push/pull) loss on 8 TRN2 NeuronCores.

Strategy (data parallel, 8 images per core):
  - The 285MB tags tensor is only touched at P*K=510 points per image, and
    only ~1/4 of those are valid (joint_img_valid & person_valid). Each
    core gathers ONLY its valid points (~1020), host-sorted by address and
    packed into C indirect-DMA window calls of 128 single-element
    descriptors (the HW contract: one index per destination partition).
    C=9 covers the valid count with >5 sigma margin; a C=32 variant
    (capacity 4096 >= all 4080 points) is compiled lazily as fallback.
  - Per-point moment contributions v, v^2 are scattered into per-image
    columns with host-built 0/1 matrices (two DVE ops per call) and
    accumulated into a PSUM [30 person, 16] tile by one PE matmul per call
    against a host-built point->person selection matrix. This replaces all
    on-device masking/reduction; everything derivable from the masks alone
    (1/cnt, per-image normalizers, invalid-person fake-mean offsets) is
    precomputed on the host in one small aux upload.
  - The pairwise push term runs in an [img part, person] layout after a
    32x32 DVE block transpose, using an invalid-person fake-mean trick so
    no pair mask is needed: push_sum = sum_ij exp(-(m'_i-m'_j)^2) - P.
    exp(-d^2) + its row sum run on the scalar engine (Square, then Exp
    with the fused accumulator).
  - Each core emits (push, pull) partials already scaled by 1/64; the host
    sums the 8 partials.
"""

import sys

import numpy as np

if "/opt/trn_rl_repo" not in sys.path:
    sys.path.insert(0, "/opt/trn_rl_repo")

from concourse import bacc, bass, mybir, tile  # noqa: E402
from concourse import bass_utils  # noqa: E402

B, P, K, H, W = 64, 30, 17, 256, 256
NCORES = 8
BPC = B // NCORES           # 8 images per core
J = BPC * K                 # 136 (img, k) columns
KHW = K * H * W
NTOT = BPC * KHW            # flat tag elements per core

C_FAST = 9                  # 1152-point capacity (valid ~1020 +- 28)
C_FULL = 32                 # 4096-point capacity (any input)

# aux30 [30, 26] f32 columns: 0:8 inv, 8:16 fakeA, 16:24 IC3,
#   24 c1 (rows 0:8), 25 c2 (rows 0:8)
AUX30W = 26

f32 = mybir.dt.float32
i32 = mybir.dt.int32
Alu = mybir.AluOpType
Act = mybir.ActivationFunctionType
AX = mybir.AxisListType


def build_nc(ncalls=C_FAST):
    nc = bacc.Bacc("TRN2", target_bir_lowering=False, debug=False,
                   num_devices=NCORES, num_swdge_queues=2)

    tags = nc.dram_tensor("tags", [NTOT, 1], f32, kind="ExternalInput")
    idx_in = nc.dram_tensor("idx", [128, ncalls], i32, kind="ExternalInput")
    # fmat columns: VA [16*ncalls] (v -> col img), VB [16*ncalls]
    # (v^2 -> col 8+img), PSEL [30*ncalls] (point -> person)
    fmat_in = nc.dram_tensor("fmat", [128, 62 * ncalls], f32,
                             kind="ExternalInput")
    aux_in = nc.dram_tensor("aux", [P, AUX30W], f32, kind="ExternalInput")
    out = nc.dram_tensor("out", [1, 2], f32, kind="ExternalOutput")

    va_off, vb_off, ps_off = 0, 16 * ncalls, 32 * ncalls

    with tile.TileContext(nc) as tc:
        with tc.tile_pool(name="sbuf", bufs=1) as pool, \
             tc.tile_pool(name="psum", bufs=1, space="PSUM") as psp:

            idxm = pool.tile([128, ncalls], i32)
            fmat = pool.tile([128, 62 * ncalls], f32)
            aux = pool.tile([P, AUX30W], f32)
            nc.sync.dma_start(out=idxm[:], in_=idx_in[:])
            nc.sync.dma_start(out=fmat[:], in_=fmat_in[:])
            nc.sync.dma_start(out=aux[:], in_=aux_in[:])

            inv = aux[:, 0:BPC]
            fakeA = aux[:, BPC:2 * BPC]
            ic3 = aux[:, 2 * BPC:3 * BPC]
            c1 = aux[0:BPC, 24:25]
            c2 = aux[0:BPC, 25:26]

            ones = pool.tile([P, 1], f32)
            nc.vector.memset(ones[:], 1.0)
            t_in = pool.tile([32, 32], f32)
            nc.vector.memset(t_in[:], 0.0)

            # ---- gather valid points + scatter-accumulate moments ----
            v4 = pool.tile([128, ncalls, 1], f32)
            rhs = pool.tile([128, ncalls, 16], f32)
            ps = psp.tile([P, 16], f32)
            for c in range(ncalls):
                call = nc.gpsimd.indirect_dma_start(
                    out=v4[:, c, :], out_offset=None, in_=tags[:],
                    in_offset=bass.IndirectOffsetOnAxis(ap=idxm[:, c:c + 1],
                                                        axis=0))
                if c % 2 == 1:
                    call.ins.queue = "qPoolDynamic1"
            for c in range(ncalls):
                v_bc = v4[:, c, :].to_broadcast([128, 16])
                rc = rhs[:, c, :]
                # rhs = (VB*v + VA) * v  ->  v at col img, v^2 at col 8+img
                nc.vector.tensor_tensor(
                    out=rc, in0=fmat[:, vb_off + 16 * c:vb_off + 16 * c + 16],
                    in1=v_bc, op=Alu.mult)
                nc.vector.tensor_tensor(
                    out=rc, in0=rc,
                    in1=fmat[:, va_off + 16 * c:va_off + 16 * c + 16],
                    op=Alu.add)
                nc.vector.tensor_tensor(out=rc, in0=rc, in1=v_bc, op=Alu.mult)
                nc.tensor.matmul(
                    out=ps[:],
                    lhsT=fmat[:, ps_off + 30 * c:ps_off + 30 * c + 30],
                    rhs=rc, start=(c == 0), stop=(c == ncalls - 1))

            s12 = pool.tile([P, 16], f32)
            nc.vector.tensor_copy(out=s12[:], in_=ps[:])
            s1 = s12[:, 0:BPC]
            s2 = s12[:, BPC:2 * BPC]

            # ---- means + fake-mean for invalid persons ----
            mean = pool.tile([P, BPC], f32)
            nc.vector.tensor_tensor(out=mean[:], in0=s1, in1=inv,
                                    op=Alu.mult)
            nc.vector.tensor_tensor(out=t_in[0:P, 0:BPC], in0=mean[:],
                                    in1=fakeA, op=Alu.add)
            t_out = pool.tile([32, 32], f32)
            nc.vector.transpose(out=t_out[:], in_=t_in[:])
            meanT = t_out[0:BPC, 0:P]          # [8, 30]

            # ---- push: s_acc[img] = sum_ij exp(-(m'_i - m'_j)^2) ----
            d = pool.tile([BPC, P, P], f32)
            nc.vector.tensor_tensor(
                out=d[:],
                in0=meanT.unsqueeze(2).to_broadcast([BPC, P, P]),
                in1=meanT.unsqueeze(1).to_broadcast([BPC, P, P]),
                op=Alu.subtract)
            sq = pool.tile([BPC, P, P], f32)
            e = pool.tile([BPC, P, P], f32)
            s_acc = pool.tile([BPC, 1], f32)
            nc.scalar.activation(out=sq[:], in_=d[:], func=Act.Square,
                                 scale=1.0)
            nc.scalar.activation(out=e[:], in_=sq[:], func=Act.Exp,
                                 scale=-1.0, accum_out=s_acc[:])

            # ---- pull: pw = (s2 - s1*mean) * inv*ninv/B ----
            sm = pool.tile([P, BPC], f32)
            dd = pool.tile([P, BPC], f32)
            pw = pool.tile([P, BPC], f32)
            pwr = pool.tile([P, 1], f32)
            nc.vector.tensor_tensor(out=sm[:], in0=s1, in1=mean[:],
                                    op=Alu.mult)
            nc.vector.tensor_tensor(out=dd[:], in0=s2, in1=sm[:],
                                    op=Alu.subtract)
            nc.vector.tensor_tensor(out=pw[:], in0=dd[:], in1=ic3,
                                    op=Alu.mult)
            nc.vector.tensor_reduce(out=pwr[:], in_=pw[:], axis=AX.X,
                                    op=Alu.add)

            # push_img = s_acc*c1 - c2  (c1 = 0.5*g/den/B, c2 = P*c1)
            pp0 = pool.tile([BPC, 1], f32)
            nc.vector.tensor_tensor(out=pp0[:], in0=s_acc[:], in1=c1,
                                    op=Alu.mult)
            nc.vector.tensor_tensor(out=pp0[:], in0=pp0[:], in1=c2,
                                    op=Alu.subtract)

            # ---- final sums: psum[0,0]=push, psum[0,1]=pull ----
            acc = psp.tile([1, 2], f32)
            nc.tensor.matmul(out=acc[:, 0:1], lhsT=pp0[:], rhs=ones[0:BPC, :],
                             start=True, stop=True)
            nc.tensor.matmul(out=acc[:, 1:2], lhsT=pwr[:], rhs=ones[:],
                             start=True, stop=True)
            res = pool.tile([1, 2], f32)
            nc.vector.tensor_copy(out=res[:], in_=acc[:])
            nc.sync.dma_start(out=out[:], in_=res[:])

    nc.compile()
    return nc


_nc_cache = {}


def _get_nc(ncalls=C_FAST):
    if ncalls not in _nc_cache:
        _nc_cache[ncalls] = build_nc(ncalls)
    return _nc_cache[ncalls]


def make_in_maps(tags, joints, jv, pv, ncalls=None):
    """Host preprocessing: per-core input dict. Returns (in_maps, ncalls)."""
    tags = np.ascontiguousarray(np.asarray(tags, dtype=np.float32))
    joints = np.asarray(joints, dtype=np.int64)
    jv = np.asarray(jv)
    pv = np.asarray(pv)

    m_all = (jv > 0) & (pv[:, :, None] > 0)            # [64, 30, 17]
    if ncalls is None:
        nv_max = max(int(m_all[c * BPC:(c + 1) * BPC].sum())
                     for c in range(NCORES))
        ncalls = C_FAST if nv_max <= 128 * C_FAST else C_FULL

    in_maps = []
    for c in range(NCORES):
        sl = slice(c * BPC, (c + 1) * BPC)
        m = m_all[sl]                                   # [8, 30, 17]
        x = joints[sl, :, :, 0]
        y = joints[sl, :, :, 1]
        img_i, p_i, k_i = np.nonzero(m)
        idx_v = (65536 * (img_i * K + k_i) + 256 * x[img_i, p_i, k_i]
                 + y[img_i, p_i, k_i]).astype(np.int64)
        order = np.argsort(idx_v, kind="stable")
        idx_v, img_i, p_i = idx_v[order], img_i[order], p_i[order]
        nv = idx_v.shape[0]
        assert nv <= 128 * ncalls, (nv, ncalls)

        t = np.arange(nv)
        q_t, c_t = t % 128, t // 128
        idxm = np.zeros((128, ncalls), dtype=np.int32)
        idxm[q_t, c_t] = idx_v
        fmat = np.zeros((128, 62 * ncalls), dtype=np.float32)
        fmat[q_t, 16 * c_t + img_i] = 1.0                       # VA
        fmat[q_t, 16 * ncalls + 16 * c_t + 8 + img_i] = 1.0     # VB
        fmat[q_t, 32 * ncalls + 30 * c_t + p_i] = 1.0           # PSEL

        cnt = m.sum(axis=2).T.astype(np.float32)        # [30, 8]
        inv = 1.0 / np.maximum(cnt, 1.0)
        fake = (cnt <= 0) * (1000.0 * (np.arange(P) + 1.0))[:, None]
        n = (cnt > 0).sum(axis=0)                       # [8]
        den = np.maximum(n * (n - 1.0), 1.0)
        c1 = 0.5 * (n > 1) / den / B
        ninv = 1.0 / np.maximum(n, 1.0)
        ic3 = inv * (ninv / B)[None, :]

        aux = np.zeros((P, AUX30W), dtype=np.float32)
        aux[:, 0:BPC] = inv
        aux[:, BPC:2 * BPC] = fake
        aux[:, 2 * BPC:3 * BPC] = ic3
        aux[0:BPC, 24] = c1
        aux[0:BPC, 25] = P * c1

        in_maps.append({
            "tags": tags[sl].reshape(NTOT, 1),
            "idx": idxm,
            "fmat": fmat,
            "aux": aux,
        })
    return in_maps, ncalls


def kernel(tags, joints, joint_img_valid, person_valid):
    in_maps, ncalls = make_in_maps(tags, joints, joint_img_valid,
                                   person_valid)
    nc = _get_nc(ncalls)
    res = bass_utils.run_bass_kernel_spmd(nc, in_maps,
                                          core_ids=list(range(NCORES)))
    outs = [np.asarray(r["out"], dtype=np.float64).reshape(2)
            for r in res.results]
    total = np.sum(outs, axis=0)
    return np.float32(total[0]), np.float32(total[1])


if __name__ == "__main__":
    rng = np.random.default_rng(0)
    t = rng.standard_normal((B, K, H, W), dtype=np.float32)
    j = rng.integers(0, H, size=(B, P, K, 2), dtype=np.int32)
    jv_ = rng.integers(0, 2, size=(B, P, K), dtype=np.int32)
    pv_ = rng.integers(0, 2, size=(B, P), dtype=np.int32)
    print(kernel(t, j, jv_, pv_))



# revision 5
# speedup vs baseline: 1.1767x; 1.1767x over previous
"""Associative-embedding (push/pull) loss on 8 TRN2 NeuronCores.

Strategy (data parallel, 8 images per core):
  - The 285MB tags tensor is only touched at P*K=510 points per image, and
    only ~1/4 of those are valid (joint_img_valid & person_valid). Each
    core gathers ONLY its valid points (~1020), host-sorted by address and
    packed into C indirect-DMA window calls of 128 single-element
    descriptors (the HW contract: one index per destination partition).
    C=9 covers the valid count with >5 sigma margin; a C=32 variant
    (capacity 4096 >= all 4080 points) is compiled lazily as fallback.
  - Per-point moment contributions v, v^2 are scattered into per-image
    columns with host-built 0/1 matrices (two DVE ops per call) and
    accumulated into a PSUM [30 person, 16] tile by one PE matmul per call
    against a host-built point->person selection matrix. This replaces all
    on-device masking/reduction; everything derivable from the masks alone
    (1/cnt, per-image normalizers, invalid-person fake-mean offsets) is
    precomputed on the host in one small aux upload.
  - The pairwise push term runs in an [img part, person] layout after a
    32x32 DVE block transpose, using an invalid-person fake-mean trick so
    no pair mask is needed: push_sum = sum_ij exp(-(m'_i-m'_j)^2) - P.
    exp(-d^2) + its row sum run on the scalar engine (Square, then Exp
    with the fused accumulator).
  - Each core emits (push, pull) partials already scaled by 1/64; the host
    sums the 8 partials.
"""

import sys

import numpy as np

if "/opt/trn_rl_repo" not in sys.path:
    sys.path.insert(0, "/opt/trn_rl_repo")

from concourse import bacc, bass, mybir, tile  # noqa: E402
from concourse import bass_utils  # noqa: E402

B, P, K, H, W = 64, 30, 17, 256, 256
NCORES = 8
BPC = B // NCORES           # 8 images per core
J = BPC * K                 # 136 (img, k) columns
KHW = K * H * W
NTOT = BPC * KHW            # flat tag elements per core

C_FAST = 9                  # 1152-point capacity (valid ~1020 +- 28)
C_FULL = 32                 # 4096-point capacity (any input)

# aux30 [30, 26] f32 columns: 0:8 inv, 8:16 fakeA, 16:24 IC3,
#   24 c1 (rows 0:8), 25 c2 (rows 0:8)
AUX30W = 26

f32 = mybir.dt.float32
i32 = mybir.dt.int32
Alu = mybir.AluOpType
Act = mybir.ActivationFunctionType
AX = mybir.AxisListType


def build_nc(ncalls=C_FAST):
    nc = bacc.Bacc("TRN2", target_bir_lowering=False, debug=False,
                   num_devices=NCORES, num_swdge_queues=2)

    tags = nc.dram_tensor("tags", [NTOT, 1], f32, kind="ExternalInput")
    idx_in = nc.dram_tensor("idx", [128, ncalls], i32, kind="ExternalInput")
    # fmat columns: VA [16*ncalls] (v -> col img), VB [16*ncalls]
    # (v^2 -> col 8+img), PSEL [30*ncalls] (point -> person)
    fmat_in = nc.dram_tensor("fmat", [128, 62 * ncalls], f32,
                             kind="ExternalInput")
    aux_in = nc.dram_tensor("aux", [P, AUX30W], f32, kind="ExternalInput")
    out = nc.dram_tensor("out", [1, 2], f32, kind="ExternalOutput")

    va_off, vb_off, ps_off = 0, 16 * ncalls, 32 * ncalls

    with tile.TileContext(nc) as tc:
        with tc.tile_pool(name="sbuf", bufs=1) as pool, \
             tc.tile_pool(name="psum", bufs=1, space="PSUM") as psp:

            idxm = pool.tile([128, ncalls], i32)
            fmat = pool.tile([128, 62 * ncalls], f32)
            aux = pool.tile([P, AUX30W], f32)
            nc.sync.dma_start(out=idxm[:], in_=idx_in[:])
            nc.sync.dma_start(out=fmat[:], in_=fmat_in[:])
            nc.sync.dma_start(out=aux[:], in_=aux_in[:])

            inv = aux[:, 0:BPC]
            fakeA = aux[:, BPC:2 * BPC]
            ic3 = aux[:, 2 * BPC:3 * BPC]
            c1 = aux[0:BPC, 24:25]
            c2 = aux[0:BPC, 25:26]

            ones = pool.tile([P, 1], f32)
            nc.vector.memset(ones[:], 1.0)
            t_in = pool.tile([32, 32], f32)
            nc.vector.memset(t_in[:], 0.0)

            # ---- gather valid points + scatter-accumulate moments ----
            v4 = pool.tile([128, ncalls, 1], f32)
            rhs = pool.tile([128, ncalls, 16], f32)
            ps = psp.tile([P, 16], f32)
            for c in range(ncalls):
                nc.gpsimd.indirect_dma_start(
                    out=v4[:, c, :], out_offset=None, in_=tags[:],
                    in_offset=bass.IndirectOffsetOnAxis(ap=idxm[:, c:c + 1],
                                                        axis=0))
            for c in range(ncalls):
                v_bc = v4[:, c, :].to_broadcast([128, 16])
                rc = rhs[:, c, :]
                # rhs = (VB*v + VA) * v  ->  v at col img, v^2 at col 8+img
                nc.vector.tensor_tensor(
                    out=rc, in0=fmat[:, vb_off + 16 * c:vb_off + 16 * c + 16],
                    in1=v_bc, op=Alu.mult)
                nc.vector.tensor_tensor(
                    out=rc, in0=rc,
                    in1=fmat[:, va_off + 16 * c:va_off + 16 * c + 16],
                    op=Alu.add)
                nc.vector.tensor_tensor(out=rc, in0=rc, in1=v_bc, op=Alu.mult)
                nc.tensor.matmul(
                    out=ps[:],
                    lhsT=fmat[:, ps_off + 30 * c:ps_off + 30 * c + 30],
                    rhs=rc, start=(c == 0), stop=(c == ncalls - 1))

            s12 = pool.tile([P, 16], f32)
            nc.vector.tensor_copy(out=s12[:], in_=ps[:])
            s1 = s12[:, 0:BPC]
            s2 = s12[:, BPC:2 * BPC]

            # ---- means + fake-mean for invalid persons ----
            mean = pool.tile([P, BPC], f32)
            nc.vector.tensor_tensor(out=mean[:], in0=s1, in1=inv,
                                    op=Alu.mult)
            nc.vector.tensor_tensor(out=t_in[0:P, 0:BPC], in0=mean[:],
                                    in1=fakeA, op=Alu.add)
            t_out = pool.tile([32, 32], f32)
            nc.vector.transpose(out=t_out[:], in_=t_in[:])
            meanT = t_out[0:BPC, 0:P]          # [8, 30]

            # ---- push: s_acc[img] = sum_ij exp(-(m'_i - m'_j)^2) ----
            d = pool.tile([BPC, P, P], f32)
            nc.vector.tensor_tensor(
                out=d[:],
                in0=meanT.unsqueeze(2).to_broadcast([BPC, P, P]),
                in1=meanT.unsqueeze(1).to_broadcast([BPC, P, P]),
                op=Alu.subtract)
            sq = pool.tile([BPC, P, P], f32)
            e = pool.tile([BPC, P, P], f32)
            s_acc = pool.tile([BPC, 1], f32)
            nc.scalar.activation(out=sq[:], in_=d[:], func=Act.Square,
                                 scale=1.0)
            nc.scalar.activation(out=e[:], in_=sq[:], func=Act.Exp,
                                 scale=-1.0, accum_out=s_acc[:])

            # ---- pull: pw = (s2 - s1*mean) * inv*ninv/B ----
            sm = pool.tile([P, BPC], f32)
            dd = pool.tile([P, BPC], f32)
            pw = pool.tile([P, BPC], f32)
            pwr = pool.tile([P, 1], f32)
            nc.vector.tensor_tensor(out=sm[:], in0=s1, in1=mean[:],
                                    op=Alu.mult)
            nc.vector.tensor_tensor(out=dd[:], in0=s2, in1=sm[:],
                                    op=Alu.subtract)
            nc.vector.tensor_tensor(out=pw[:], in0=dd[:], in1=ic3,
                                    op=Alu.mult)
            nc.vector.tensor_reduce(out=pwr[:], in_=pw[:], axis=AX.X,
                                    op=Alu.add)

            # push_img = s_acc*c1 - c2  (c1 = 0.5*g/den/B, c2 = P*c1)
            pp0 = pool.tile([BPC, 1], f32)
            nc.vector.tensor_tensor(out=pp0[:], in0=s_acc[:], in1=c1,
                                    op=Alu.mult)
            nc.vector.tensor_tensor(out=pp0[:], in0=pp0[:], in1=c2,
                                    op=Alu.subtract)

            # ---- final sums: psum[0,0]=push, psum[0,1]=pull ----
            acc = psp.tile([1, 2], f32)
            nc.tensor.matmul(out=acc[:, 0:1], lhsT=pp0[:], rhs=ones[0:BPC, :],
                             start=True, stop=True)
            nc.tensor.matmul(out=acc[:, 1:2], lhsT=pwr[:], rhs=ones[:],
                             start=True, stop=True)
            res = pool.tile([1, 2], f32)
            nc.vector.tensor_copy(out=res[:], in_=acc[:])
            nc.sync.dma_start(out=out[:], in_=res[:])

    nc.compile()
    return nc


_nc_cache = {}


def _get_nc(ncalls=C_FAST):
    if ncalls not in _nc_cache:
        _nc_cache[ncalls] = build_nc(ncalls)
    return _nc_cache[ncalls]


def make_in_maps(tags, joints, jv, pv, ncalls=None):
    """Host preprocessing: per-core input dict. Returns (in_maps, ncalls)."""
    tags = np.ascontiguousarray(np.asarray(tags, dtype=np.float32))
    joints = np.asarray(joints, dtype=np.int64)
    jv = np.asarray(jv)
    pv = np.asarray(pv)

    m_all = (jv > 0) & (pv[:, :, None] > 0)            # [64, 30, 17]
    if ncalls is None:
        nv_max = max(int(m_all[c * BPC:(c + 1) * BPC].sum())
                     for c in range(NCORES))
        ncalls = C_FAST if nv_max <= 128 * C_FAST else C_FULL

    in_maps = []
    for c in range(NCORES):
        sl = slice(c * BPC, (c + 1) * BPC)
        m = m_all[sl]                                   # [8, 30, 17]
        x = joints[sl, :, :, 0]
        y = joints[sl, :, :, 1]
        img_i, p_i, k_i = np.nonzero(m)
        idx_v = (65536 * (img_i * K + k_i) + 256 * x[img_i, p_i, k_i]
                 + y[img_i, p_i, k_i]).astype(np.int64)
        order = np.argsort(idx_v, kind="stable")
        idx_v, img_i, p_i = idx_v[order], img_i[order], p_i[order]
        nv = idx_v.shape[0]
        assert nv <= 128 * ncalls, (nv, ncalls)

        t = np.arange(nv)
        q_t, c_t = t % 128, t // 128
        idxm = np.zeros((128, ncalls), dtype=np.int32)
        idxm[q_t, c_t] = idx_v
        fmat = np.zeros((128, 62 * ncalls), dtype=np.float32)
        fmat[q_t, 16 * c_t + img_i] = 1.0                       # VA
        fmat[q_t, 16 * ncalls + 16 * c_t + 8 + img_i] = 1.0     # VB
        fmat[q_t, 32 * ncalls + 30 * c_t + p_i] = 1.0           # PSEL

        cnt = m.sum(axis=2).T.astype(np.float32)        # [30, 8]
        inv = 1.0 / np.maximum(cnt, 1.0)
        fake = (cnt <= 0) * (1000.0 * (np.arange(P) + 1.0))[:, None]
        n = (cnt > 0).sum(axis=0)                       # [8]
        den = np.maximum(n * (n - 1.0), 1.0)
        c1 = 0.5 * (n > 1) / den / B
        ninv = 1.0 / np.maximum(n, 1.0)
        ic3 = inv * (ninv / B)[None, :]

        aux = np.zeros((P, AUX30W), dtype=np.float32)
        aux[:, 0:BPC] = inv
        aux[:, BPC:2 * BPC] = fake
        aux[:, 2 * BPC:3 * BPC] = ic3
        aux[0:BPC, 24] = c1
        aux[0:BPC, 25] = P * c1

        in_maps.append({
            "tags": tags[sl].reshape(NTOT, 1),
            "idx": idxm,
            "fmat": fmat,
            "aux": aux,
        })
    return in_maps, ncalls


def kernel(tags, joints, joint_img_valid, person_valid):
    in_maps, ncalls = make_in_maps(tags, joints, joint_img_valid,
                                   person_valid)
    nc = _get_nc(ncalls)
    res = bass_utils.run_bass_kernel_spmd(nc, in_maps,
                                          core_ids=list(range(NCORES)))
    outs = [np.asarray(r["out"], dtype=np.float64).reshape(2)
            for r in res.results]
    total = np.sum(outs, axis=0)
    return np.float32(total[0]), np.float32(total[1])


if __name__ == "__main__":
    rng = np.random.default_rng(0)
    t = rng.standard_normal((B, K, H, W), dtype=np.float32)
    j = rng.integers(0, H, size=(B, P, K, 2), dtype=np.int32)
    jv_ = rng.integers(0, 2, size=(B, P, K), dtype=np.int32)
    pv_ = rng.integers(0, 2, size=(B, P), dtype=np.int32)
    print(kernel(t, j, jv_, pv_))



# revision 7
# speedup vs baseline: 1.3422x; 1.1407x over previous
"""Associative-embedding (push/pull) loss on 8 TRN2 NeuronCores.

Strategy (data parallel, 8 images per core, balanced):
  - The 285MB tags tensor is only touched at P*K=510 points per image, and
    only ~1/4 of those are valid. Images are BIN-PACKED onto cores so every
    core's valid-point count fits exactly C=8 indirect-DMA windows of 128
    single-element descriptors (the HW contract: one index per destination
    partition). Larger counts fall back to a lazily compiled wider variant.
  - Per 128-point block, one fp16 PE matmul scatter-accumulates the moment
    pair (v, v^2) into a PSUM tile s12E[120, 4] laid out as
    (rho=(g,p), (moment, b)) with img = 4*b + g. The one-hot point->rho
    selection matrix and the img_hi (b) mask are host-built fp16 uploads;
    on-device per block: an fp16 copy, a square, and a tiny outer mask mult.
  - Pull runs entirely in the [120, 2] layout (4 DVE ops + one ones-matmul).
  - Push uses an invalid-person fake-mean offset (no pair mask) in a
    [120, 60] layout: a host-built group-select fp16 matmul replicates the
    per-image mean row into all 30 partition rows of its image group, so
    the pairwise difference, Square, and Exp all run 120 partitions wide.
  - Each core emits (push, pull) partials already scaled by 1/64; the host
    sums the 8 partials.
"""

import sys

import numpy as np

if "/opt/trn_rl_repo" not in sys.path:
    sys.path.insert(0, "/opt/trn_rl_repo")

from concourse import bacc, bass, mybir, tile  # noqa: E402
from concourse import bass_utils  # noqa: E402

B, P, K, H, W = 64, 30, 17, 256, 256
NCORES = 8
BPC = B // NCORES           # 8 images per core
NTOT = BPC * K * H * W      # flat tag elements per core
C_FAST = 8                  # 1024-point capacity (valid ~1019 after balance)

R = 120                     # rho = g*30 + p, g = img%4, p = person
PSW = 122                   # per-call pselh cols: 120 one-hot + 2 b-mask
AUXF_W = 14                 # invE 0:2, fakeE 2:4, ic3E 4:6, IMGSEL 6:10,
#                             c1E 10:12 (rows 0:4), c2E 12:14 (rows 0:4)
AUXH_W = 150                # IDPh 0:30, GSELh 30:150

f32 = mybir.dt.float32
f16 = mybir.dt.float16
i32 = mybir.dt.int32
Alu = mybir.AluOpType
Act = mybir.ActivationFunctionType
AX = mybir.AxisListType


def build_nc(ncalls=C_FAST):
    nc = bacc.Bacc("TRN2", target_bir_lowering=False, debug=False,
                   num_devices=NCORES)

    tags = nc.dram_tensor("tags", [NTOT, 1], f32, kind="ExternalInput")
    idx_in = nc.dram_tensor("idx", [128, ncalls], i32, kind="ExternalInput")
    psel_in = nc.dram_tensor("pselh", [128, PSW * ncalls], f16,
                             kind="ExternalInput")
    auxf_in = nc.dram_tensor("auxf", [R, AUXF_W], f32, kind="ExternalInput")
    auxh_in = nc.dram_tensor("auxh", [R, AUXH_W], f16, kind="ExternalInput")
    out = nc.dram_tensor("out", [1, 2], f32, kind="ExternalOutput")

    with tile.TileContext(nc) as tc:
        with tc.tile_pool(name="sbuf", bufs=1) as pool, \
             tc.tile_pool(name="psum", bufs=1, space="PSUM") as psp:

            idxm = pool.tile([128, ncalls], i32)
            pselh = pool.tile([128, PSW * ncalls], f16)
            auxf = pool.tile([R, AUXF_W], f32)
            auxh = pool.tile([R, AUXH_W], f16)
            nc.sync.dma_start(out=idxm[:], in_=idx_in[:])
            nc.sync.dma_start(out=pselh[:], in_=psel_in[:])
            nc.sync.dma_start(out=auxf[:], in_=auxf_in[:])
            nc.sync.dma_start(out=auxh[:], in_=auxh_in[:])

            invE = auxf[:, 0:2]
            fakeE = auxf[:, 2:4]
            ic3E = auxf[:, 4:6]
            imgsel = auxf[:, 6:10]
            c1E = auxf[0:4, 10:12]
            c2E = auxf[0:4, 12:14]
            idph = auxh[:, 0:30]
            gselh = auxh[:, 30:150]

            ones = pool.tile([R, 1], f32)
            nc.vector.memset(ones[:], 1.0)

            # ---- gather valid points (one idx per partition per call) ----
            v4 = pool.tile([128, ncalls, 1], f32)
            for c in range(ncalls):
                nc.gpsimd.indirect_dma_start(
                    out=v4[:, c, :], out_offset=None, in_=tags[:],
                    in_offset=bass.IndirectOffsetOnAxis(ap=idxm[:, c:c + 1],
                                                        axis=0))

            # ---- moments: s12E[rho, (m, b)] += psel^T (vv * bmask) ----
            vv = pool.tile([128, ncalls, 2], f16)
            vvb = pool.tile([128, ncalls, 2, 2], f16)
            s12p = psp.tile([R, 4], f32)
            for c in range(ncalls):
                vc = v4[:, c, :]
                nc.vector.tensor_copy(out=vv[:, c, 0:1], in_=vc)
                nc.vector.tensor_tensor(out=vv[:, c, 1:2], in0=vc, in1=vc,
                                        op=Alu.mult)
                bm = pselh[:, PSW * c + 120:PSW * c + 122]
                nc.vector.tensor_tensor(
                    out=vvb[:, c, :, :],
                    in0=vv[:, c, :].unsqueeze(2).to_broadcast([128, 2, 2]),
                    in1=bm.unsqueeze(1).to_broadcast([128, 2, 2]),
                    op=Alu.mult)
                nc.tensor.matmul(
                    out=s12p[:],
                    lhsT=pselh[:, PSW * c:PSW * c + 120],
                    rhs=vvb[:, c, :, :],
                    start=(c == 0), stop=(c == ncalls - 1))

            # ---- means + pull in [120, 2] ----
            meanE = pool.tile([R, 2], f32)
            sm = pool.tile([R, 2], f32)
            dd = pool.tile([R, 2], f32)
            pw = pool.tile([R, 2], f32)
            nc.vector.tensor_tensor(out=meanE[:], in0=s12p[:, 0:2], in1=invE,
                                    op=Alu.mult)
            nc.vector.tensor_tensor(out=sm[:], in0=s12p[:, 0:2], in1=meanE[:],
                                    op=Alu.mult)
            nc.vector.tensor_tensor(out=dd[:], in0=s12p[:, 2:4], in1=sm[:],
                                    op=Alu.subtract)
            nc.vector.tensor_tensor(out=pw[:], in0=dd[:], in1=ic3E,
                                    op=Alu.mult)
            pullp = psp.tile([1, 2], f32)
            nc.tensor.matmul(out=pullp[:], lhsT=ones[:], rhs=pw[:],
                             start=True, stop=True)
            res = pool.tile([1, 2], f32)
            nc.vector.tensor_reduce(out=res[:, 1:2], in_=pullp[:], axis=AX.X,
                                    op=Alu.add)

            # ---- push in [120, 60]: fake means kill invalid persons ----
            meanF16 = pool.tile([R, 2], f16)
            meanFr = pool.tile([R, 2], f32)
            nc.vector.tensor_tensor(out=meanF16[:], in0=meanE[:], in1=fakeE,
                                    op=Alu.add)
            nc.vector.tensor_copy(out=meanFr[:], in_=meanF16[:])
            rhsm = pool.tile([R, 2, 30], f16)
            nc.vector.tensor_tensor(
                out=rhsm[:],
                in0=meanF16[:].unsqueeze(2).to_broadcast([R, 2, 30]),
                in1=idph.unsqueeze(1).to_broadcast([R, 2, 30]),
                op=Alu.mult)
            mrep = psp.tile([R, 2, 30], f32)
            nc.tensor.matmul(out=mrep[:], lhsT=gselh, rhs=rhsm[:],
                             start=True, stop=True)
            d = pool.tile([R, 2, 30], f32)
            nc.vector.tensor_tensor(
                out=d[:],
                in0=meanFr[:].unsqueeze(2).to_broadcast([R, 2, 30]),
                in1=mrep[:], op=Alu.subtract)
            sq = pool.tile([R, 2, 30], f32)
            e = pool.tile([R, 2, 30], f32)
            nc.scalar.activation(out=sq[:], in_=d[:], func=Act.Square,
                                 scale=1.0)
            nc.scalar.activation(out=e[:], in_=sq[:], func=Act.Exp,
                                 scale=-1.0)
            sacc = pool.tile([R, 2], f32)
            nc.vector.tensor_reduce(out=sacc[:], in_=e[:], axis=AX.X,
                                    op=Alu.add)
            pacc = psp.tile([4, 2], f32)
            nc.tensor.matmul(out=pacc[:], lhsT=imgsel, rhs=sacc[:],
                             start=True, stop=True)
            pp = pool.tile([4, 2], f32)
            nc.vector.tensor_tensor(out=pp[:], in0=pacc[:], in1=c1E,
                                    op=Alu.mult)
            nc.vector.tensor_tensor(out=pp[:], in0=pp[:], in1=c2E,
                                    op=Alu.subtract)
            pushp = psp.tile([1, 2], f32)
            nc.tensor.matmul(out=pushp[:], lhsT=ones[0:4, :], rhs=pp[:],
                             start=True, stop=True)
            nc.vector.tensor_reduce(out=res[:, 0:1], in_=pushp[:], axis=AX.X,
                                    op=Alu.add)

            nc.sync.dma_start(out=out[:], in_=res[:])

    nc.compile()
    return nc


_nc_cache = {}


def _get_nc(ncalls=C_FAST):
    if ncalls not in _nc_cache:
        _nc_cache[ncalls] = build_nc(ncalls)
    return _nc_cache[ncalls]


def _balance_images(per_img):
    """LPT + swap refinement: 8 bins x 8 images, minimize max point total."""
    order = np.argsort(-per_img)
    bins = [[] for _ in range(NCORES)]
    tot = [0] * NCORES
    for i in order:
        cands = [b for b in range(NCORES) if len(bins[b]) < BPC]
        b = min(cands, key=lambda x: tot[x])
        bins[b].append(int(i))
        tot[b] += int(per_img[i])
    for _ in range(1000):
        hi = int(np.argmax(tot))
        best = None
        for lo in range(NCORES):
            if lo == hi:
                continue
            for ii, a in enumerate(bins[hi]):
                for jj, b2 in enumerate(bins[lo]):
                    delta = int(per_img[a]) - int(per_img[b2])
                    if delta > 0 and max(tot[hi] - delta,
                                         tot[lo] + delta) < tot[hi]:
                        best = (lo, ii, jj, delta)
                        break
                if best:
                    break
            if best:
                break
        if not best:
            break
        lo, ii, jj, delta = best
        bins[hi][ii], bins[lo][jj] = bins[lo][jj], bins[hi][ii]
        tot[hi] -= delta
        tot[lo] += delta
    return bins, max(tot)


def make_in_maps(tags, joints, jv, pv, ncalls=None):
    """Host preprocessing: per-core input dict. Returns (in_maps, ncalls)."""
    tags = np.asarray(tags, dtype=np.float32).reshape(B, K * H * W)
    joints = np.asarray(joints, dtype=np.int64)
    jv = np.asarray(jv)
    pv = np.asarray(pv)

    m_all = (jv > 0) & (pv[:, :, None] > 0)            # [64, 30, 17]
    bins, mx = _balance_images(m_all.sum((1, 2)))
    if ncalls is None:
        ncalls = max(C_FAST, -(-mx // 128))

    x_all = joints[:, :, :, 0]
    y_all = joints[:, :, :, 1]

    in_maps = []
    for core in range(NCORES):
        imgs = bins[core]
        m = m_all[imgs]                                 # [8, 30, 17]
        x = x_all[imgs]
        y = y_all[imgs]
        li_i, p_i, k_i = np.nonzero(m)
        addr = (65536 * (li_i * K + k_i) + 256 * x[li_i, p_i, k_i]
                + y[li_i, p_i, k_i]).astype(np.int64)
        order = np.argsort(addr, kind="stable")
        addr, li_i, p_i = addr[order], li_i[order], p_i[order]
        nv = addr.shape[0]
        assert nv <= 128 * ncalls, (nv, ncalls)

        t = np.arange(nv)
        q_t, c_t = t % 128, t // 128
        idxm = np.zeros((128, ncalls), dtype=np.int32)
        idxm[q_t, c_t] = addr
        g_i = li_i % 4
        b_i = li_i // 4
        pselh = np.zeros((128, PSW * ncalls), dtype=np.float16)
        pselh[q_t, PSW * c_t + g_i * 30 + p_i] = 1.0
        pselh[q_t, PSW * c_t + 120 + b_i] = 1.0

        cnt = m.sum(axis=2).astype(np.float32)          # [8 img, 30 p]
        n = (cnt > 0).sum(axis=1).astype(np.float32)    # [8]
        ninv = 1.0 / np.maximum(n, 1.0)
        den = np.maximum(n * (n - 1.0), 1.0)
        c1 = 0.5 * (n > 1) / den / B

        rho_g = np.arange(R) // 30                      # g
        rho_p = np.arange(R) % 30                       # p
        bb = np.arange(2)
        img_rb = rho_g[:, None] + 4 * bb[None, :]       # [120, 2] img idx
        cnt_rb = cnt[img_rb, rho_p[:, None]]            # [120, 2]
        auxf = np.zeros((R, AUXF_W), dtype=np.float32)
        auxf[:, 0:2] = 1.0 / np.maximum(cnt_rb, 1.0)
        auxf[:, 2:4] = (cnt_rb <= 0) * (1000.0 * (rho_p[:, None] + 1.0))
        auxf[:, 4:6] = auxf[:, 0:2] * (ninv[img_rb] / B)
        auxf[:, 6:10] = (rho_g[:, None] == np.arange(4)[None, :])
        img_gb = np.arange(4)[:, None] + 4 * bb[None, :]    # [4, 2]
        auxf[0:4, 10:12] = c1[img_gb]
        auxf[0:4, 12:14] = P * c1[img_gb]

        auxh = np.zeros((R, AUXH_W), dtype=np.float16)
        auxh[:, 0:30] = (rho_p[:, None] == np.arange(30)[None, :])
        auxh[:, 30:150] = (rho_g[:, None] == rho_g[None, :])

        in_maps.append({
            "tags": np.ascontiguousarray(tags[imgs]).reshape(NTOT, 1),
            "idx": idxm,
            "pselh": pselh,
            "auxf": auxf,
            "auxh": auxh,
        })
    return in_maps, ncalls


def kernel(tags, joints, joint_img_valid, person_valid):
    in_maps, ncalls = make_in_maps(tags, joints, joint_img_valid,
                                   person_valid)
    nc = _get_nc(ncalls)
    res = bass_utils.run_bass_kernel_spmd(nc, in_maps,
                                          core_ids=list(range(NCORES)))
    outs = [np.asarray(r["out"], dtype=np.float64).reshape(2)
            for r in res.results]
    total = np.sum(outs, axis=0)
    return np.float32(total[0]), np.float32(total[1])


if __name__ == "__main__":
    rng = np.random.default_rng(0)
    t = rng.standard_normal((B, K, H, W), dtype=np.float32)
    j = rng.integers(0, H, size=(B, P, K, 2), dtype=np.int32)
    jv_ = rng.integers(0, 2, size=(B, P, K), dtype=np.int32)
    pv_ = rng.integers(0, 2, size=(B, P), dtype=np.int32)
    print(kernel(t, j, jv_, pv_))


# revision 15
# speedup vs baseline: 1.3462x; 1.0030x over previous
"""Associative-embedding (push/pull) loss on 8 TRN2 NeuronCores.

Strategy (data parallel, 8 images per core, balanced):
  - The 285MB tags tensor is only touched at P*K=510 points per image, and
    only ~1/4 of those are valid. Images are BIN-PACKED onto cores so every
    core's valid-point count fits exactly C=8 indirect-DMA windows of 128
    single-element descriptors (the HW contract: one index per destination
    partition). Larger counts fall back to a lazily compiled wider variant.
  - Per 128-point block, one fp16 PE matmul scatter-accumulates the moment
    pair (v, v^2) into a PSUM tile s12E[120, 4] laid out as
    (rho=(g,p), (moment, b)) with img = 4*b + g. The one-hot point->rho
    selection matrix and the img_hi (b) mask are host-built fp16 uploads;
    on-device per block: an fp16 copy, a square, and a tiny outer mask mult.
  - Pull runs entirely in the [120, 2] layout (4 DVE ops + one ones-matmul).
  - Push uses an invalid-person fake-mean offset (no pair mask) in a
    [120, 60] layout: a host-built group-select fp16 matmul replicates the
    per-image mean row into all 30 partition rows of its image group, so
    the pairwise difference, Square, and Exp all run 120 partitions wide.
  - Each core emits (push, pull) partials already scaled by 1/64; the host
    sums the 8 partials.
"""

import sys

import numpy as np

if "/opt/trn_rl_repo" not in sys.path:
    sys.path.insert(0, "/opt/trn_rl_repo")

from concourse import bacc, bass, mybir, tile  # noqa: E402
from concourse import bass_utils  # noqa: E402

B, P, K, H, W = 64, 30, 17, 256, 256
NCORES = 8
BPC = B // NCORES           # 8 images per core
NTOT = BPC * K * H * W      # flat tag elements per core
C_FAST = 8                  # 1024-point capacity (valid ~1019 after balance)

R = 120                     # rho = g*30 + p, g = img%4, p = person
PSW = 120                   # per-call pselh cols: 120 one-hot
AUXF_W = 8                  # invE 0:2, fakeE 2:4, ic3E 4:6, c1R 6:8
AUXH_W = 150                # IDPh 0:30, GSELh 30:150

f32 = mybir.dt.float32
f16 = mybir.dt.float16
i32 = mybir.dt.int32
Alu = mybir.AluOpType
Act = mybir.ActivationFunctionType
AX = mybir.AxisListType


def build_nc(ncalls=C_FAST):
    nc = bacc.Bacc("TRN2", target_bir_lowering=False, debug=False,
                   num_devices=NCORES)

    tags = nc.dram_tensor("tags", [NTOT, 1], f32, kind="ExternalInput")
    idx_in = nc.dram_tensor("idx", [128, ncalls], i32, kind="ExternalInput")
    psel_in = nc.dram_tensor("pselh", [128, PSW * ncalls], f16,
                             kind="ExternalInput")
    bmf_in = nc.dram_tensor("bmf", [128, 2 * ncalls], f32,
                            kind="ExternalInput")
    auxf_in = nc.dram_tensor("auxf", [R, AUXF_W], f32, kind="ExternalInput")
    auxh_in = nc.dram_tensor("auxh", [R, AUXH_W], f16, kind="ExternalInput")
    out = nc.dram_tensor("out", [1, 4], f32, kind="ExternalOutput")

    with tile.TileContext(nc) as tc:
        with tc.tile_pool(name="sbuf", bufs=1) as pool, \
             tc.tile_pool(name="psum", bufs=1, space="PSUM") as psp:

            idxm = pool.tile([128, ncalls], i32)
            pselh = pool.tile([128, PSW * ncalls], f16)
            bmf = pool.tile([128, ncalls, 2], f32)
            auxf = pool.tile([R, AUXF_W], f32)
            auxh = pool.tile([R, AUXH_W], f16)
            nc.sync.dma_start(out=idxm[:], in_=idx_in[:])
            nc.sync.dma_start(out=pselh[:], in_=psel_in[:])
            nc.sync.dma_start(out=bmf[:], in_=bmf_in[:])
            nc.sync.dma_start(out=auxf[:], in_=auxf_in[:])
            nc.sync.dma_start(out=auxh[:], in_=auxh_in[:])

            invE = auxf[:, 0:2]
            fakeE = auxf[:, 2:4]
            ic3E = auxf[:, 4:6]
            c1R = auxf[:, 6:8]
            idph = auxh[:, 0:30]
            gselh = auxh[:, 30:150]

            ones = pool.tile([R, 1], f32)
            nc.vector.memset(ones[:], 1.0)

            # ---- gather valid points (one idx per partition per call) ----
            v4 = pool.tile([128, ncalls, 1], f32)
            for c in range(ncalls):
                nc.gpsimd.indirect_dma_start(
                    out=v4[:, c, :], out_offset=None, in_=tags[:],
                    in_offset=bass.IndirectOffsetOnAxis(ap=idxm[:, c:c + 1],
                                                        axis=0))

            # ---- moments: s12E[rho, (m, b)] += psel^T (v^m * bmask) ----
            q2 = pool.tile([128, ncalls, 1], f32)
            vvb = pool.tile([128, ncalls, 2, 2], f16)
            s12p = psp.tile([R, 4], f32)
            for c in range(ncalls):
                vc = v4[:, c, :]
                nc.vector.tensor_tensor(out=q2[:, c, :], in0=vc, in1=vc,
                                        op=Alu.mult)
                nc.vector.tensor_tensor(
                    out=vvb[:, c, 0, :], in0=vc.to_broadcast([128, 2]),
                    in1=bmf[:, c, :], op=Alu.mult)
                nc.vector.tensor_tensor(
                    out=vvb[:, c, 1, :],
                    in0=q2[:, c, :].to_broadcast([128, 2]),
                    in1=bmf[:, c, :], op=Alu.mult)
                nc.tensor.matmul(
                    out=s12p[:],
                    lhsT=pselh[:, PSW * c:PSW * c + 120],
                    rhs=vvb[:, c, :, :],
                    start=(c == 0), stop=(c == ncalls - 1))

            # ---- means + pull in [120, 2]; pwq packs (pull, push) cols ----
            meanE = pool.tile([R, 2], f32)
            sm = pool.tile([R, 2], f32)
            dd = pool.tile([R, 2], f32)
            pwq = pool.tile([R, 4], f32)
            nc.vector.tensor_tensor(out=meanE[:], in0=s12p[:, 0:2], in1=invE,
                                    op=Alu.mult)
            nc.vector.tensor_tensor(out=sm[:], in0=s12p[:, 0:2], in1=meanE[:],
                                    op=Alu.mult)
            nc.vector.tensor_tensor(out=dd[:], in0=s12p[:, 2:4], in1=sm[:],
                                    op=Alu.subtract)
            nc.vector.tensor_tensor(out=pwq[:, 0:2], in0=dd[:], in1=ic3E,
                                    op=Alu.mult)

            # ---- push in [120, 60]: fake means kill invalid persons ----
            meanF16 = pool.tile([R, 2], f16)
            meanFr = pool.tile([R, 2], f32)
            nc.vector.tensor_tensor(out=meanF16[:], in0=meanE[:], in1=fakeE,
                                    op=Alu.add)
            nc.vector.tensor_copy(out=meanFr[:], in_=meanF16[:])
            rhsm = pool.tile([R, 2, 30], f16)
            nc.vector.tensor_tensor(
                out=rhsm[:],
                in0=meanF16[:].unsqueeze(2).to_broadcast([R, 2, 30]),
                in1=idph.unsqueeze(1).to_broadcast([R, 2, 30]),
                op=Alu.mult)
            mrep = psp.tile([R, 2, 30], f32)
            nc.tensor.matmul(out=mrep[:], lhsT=gselh, rhs=rhsm[:],
                             start=True, stop=True)
            d = pool.tile([R, 2, 30], f32)
            nc.vector.tensor_tensor(
                out=d[:],
                in0=meanFr[:].unsqueeze(2).to_broadcast([R, 2, 30]),
                in1=mrep[:], op=Alu.subtract)
            sq = pool.tile([R, 2, 30], f32)
            e = pool.tile([R, 2, 30], f32)
            nc.vector.tensor_tensor(out=sq[:], in0=d[:], in1=d[:],
                                    op=Alu.mult)
            nc.scalar.activation(out=e[:], in_=sq[:], func=Act.Exp,
                                 scale=-1.0)
            sacc = pool.tile([R, 2], f32)
            nc.vector.tensor_reduce(out=sacc[:], in_=e[:], axis=AX.X,
                                    op=Alu.add)
            nc.vector.tensor_tensor(out=pwq[:, 2:4], in0=sacc[:], in1=c1R,
                                    op=Alu.mult)
            finp = psp.tile([1, 4], f32)
            nc.tensor.matmul(out=finp[:], lhsT=ones[:], rhs=pwq[:],
                             start=True, stop=True)
            res = pool.tile([1, 4], f32)
            nc.vector.tensor_copy(out=res[:], in_=finp[:])
            nc.sync.dma_start(out=out[:], in_=res[:])

    nc.compile()
    return nc


_nc_cache = {}


def _get_nc(ncalls=C_FAST):
    if ncalls not in _nc_cache:
        _nc_cache[ncalls] = build_nc(ncalls)
    return _nc_cache[ncalls]


def _balance_images(per_img):
    """LPT + swap refinement: 8 bins x 8 images, minimize max point total."""
    order = np.argsort(-per_img)
    bins = [[] for _ in range(NCORES)]
    tot = [0] * NCORES
    for i in order:
        cands = [b for b in range(NCORES) if len(bins[b]) < BPC]
        b = min(cands, key=lambda x: tot[x])
        bins[b].append(int(i))
        tot[b] += int(per_img[i])
    for _ in range(1000):
        hi = int(np.argmax(tot))
        best = None
        for lo in range(NCORES):
            if lo == hi:
                continue
            for ii, a in enumerate(bins[hi]):
                for jj, b2 in enumerate(bins[lo]):
                    delta = int(per_img[a]) - int(per_img[b2])
                    if delta > 0 and max(tot[hi] - delta,
                                         tot[lo] + delta) < tot[hi]:
                        best = (lo, ii, jj, delta)
                        break
                if best:
                    break
            if best:
                break
        if not best:
            break
        lo, ii, jj, delta = best
        bins[hi][ii], bins[lo][jj] = bins[lo][jj], bins[hi][ii]
        tot[hi] -= delta
        tot[lo] += delta
    return bins, max(tot)


def make_in_maps(tags, joints, jv, pv, ncalls=None):
    """Host preprocessing: per-core input dict. Returns (in_maps, ncalls)."""
    tags = np.asarray(tags, dtype=np.float32).reshape(B, K * H * W)
    joints = np.asarray(joints, dtype=np.int64)
    jv = np.asarray(jv)
    pv = np.asarray(pv)

    m_all = (jv > 0) & (pv[:, :, None] > 0)            # [64, 30, 17]
    bins, mx = _balance_images(m_all.sum((1, 2)))
    if ncalls is None:
        ncalls = max(C_FAST, -(-mx // 128))

    x_all = joints[:, :, :, 0]
    y_all = joints[:, :, :, 1]

    in_maps = []
    for core in range(NCORES):
        imgs = bins[core]
        m = m_all[imgs]                                 # [8, 30, 17]
        x = x_all[imgs]
        y = y_all[imgs]
        li_i, p_i, k_i = np.nonzero(m)
        addr = (65536 * (li_i * K + k_i) + 256 * x[li_i, p_i, k_i]
                + y[li_i, p_i, k_i]).astype(np.int64)
        order = np.argsort(addr, kind="stable")
        addr, li_i, p_i = addr[order], li_i[order], p_i[order]
        nv = addr.shape[0]
        assert nv <= 128 * ncalls, (nv, ncalls)

        t = np.arange(nv)
        q_t, c_t = t % 128, t // 128
        idxm = np.zeros((128, ncalls), dtype=np.int32)
        idxm[q_t, c_t] = addr
        g_i = li_i % 4
        b_i = li_i // 4
        pselh = np.zeros((128, PSW * ncalls), dtype=np.float16)
        pselh[q_t, PSW * c_t + g_i * 30 + p_i] = 1.0
        bmf = np.zeros((128, 2 * ncalls), dtype=np.float32)
        bmf[q_t, 2 * c_t + b_i] = 1.0

        cnt = m.sum(axis=2).astype(np.float32)          # [8 img, 30 p]
        n = (cnt > 0).sum(axis=1).astype(np.float32)    # [8]
        ninv = 1.0 / np.maximum(n, 1.0)
        den = np.maximum(n * (n - 1.0), 1.0)
        c1 = 0.5 * (n > 1) / den / B

        rho_g = np.arange(R) // 30                      # g
        rho_p = np.arange(R) % 30                       # p
        bb = np.arange(2)
        img_rb = rho_g[:, None] + 4 * bb[None, :]       # [120, 2] img idx
        cnt_rb = cnt[img_rb, rho_p[:, None]]            # [120, 2]
        auxf = np.zeros((R, AUXF_W), dtype=np.float32)
        auxf[:, 0:2] = 1.0 / np.maximum(cnt_rb, 1.0)
        auxf[:, 2:4] = (cnt_rb <= 0) * (1000.0 * (rho_p[:, None] + 1.0))
        auxf[:, 4:6] = auxf[:, 0:2] * (ninv[img_rb] / B)
        auxf[:, 6:8] = c1[img_rb]                       # c1R
        c2sum = float(P * c1.sum())

        auxh = np.zeros((R, AUXH_W), dtype=np.float16)
        auxh[:, 0:30] = (rho_p[:, None] == np.arange(30)[None, :])
        auxh[:, 30:150] = (rho_g[:, None] == rho_g[None, :])

        in_maps.append({
            "tags": np.ascontiguousarray(tags[imgs]).reshape(NTOT, 1),
            "idx": idxm,
            "pselh": pselh,
            "bmf": bmf,
            "auxf": auxf,
            "auxh": auxh,
            "_c2sum": c2sum,
        })
    return in_maps, ncalls


def kernel(tags, joints, joint_img_valid, person_valid):
    in_maps, ncalls = make_in_maps(tags, joints, joint_img_valid,
                                   person_valid)
    c2sums = [im.pop("_c2sum") for im in in_maps]
    nc = _get_nc(ncalls)
    res = bass_utils.run_bass_kernel_spmd(nc, in_maps,
                                          core_ids=list(range(NCORES)))
    push = pull = 0.0
    for r, c2s in zip(res.results, c2sums):
        o = np.asarray(r["out"], dtype=np.float64).reshape(4)
        pull += o[0] + o[1]
        push += o[2] + o[3] - c2s
    return np.float32(push), np.float32(pull)


if __name__ == "__main__":
    rng = np.random.default_rng(0)
    t = rng.standard_normal((B, K, H, W), dtype=np.float32)
    j = rng.integers(0, H, size=(B, P, K, 2), dtype=np.int32)
    jv_ = rng.integers(0, 2, size=(B, P, K), dtype=np.int32)
    pv_ = rng.integers(0, 2, size=(B, P), dtype=np.int32)
    print(kernel(t, j, jv_, pv_))
